# revision 1
# baseline (speedup 1.0000x reference)
"""AdaptiveCenterLoss on 8 TRN2 NeuronCores.

loss = mean_i ||features[i] - centers[labels[i]]||^2
     with B=131072, D=256, C=1000.

Strategy (data-parallel, memory-bound):
  - host-side, sort rows by label and pack them into one-label blocks;
    partial blocks are padded with rows equal to that class's center
    (contributing exactly 0 to the sum).  Each class's bulk goes into
    16-row blocks; a remainder of <= 8 rows goes into an 8-row block in
    trailing 8-slot tiles (halves the padding vs all-16 blocks).
  - shard the blocks across 8 cores x 128 partitions, one block per
    partition per tile; ONE [128,1]-index indirect DMA per tile gathers
    the 128 needed center rows (the HW DGE consumes one index per
    partition per call and costs ~10ns/descriptor of Q7 software time,
    so per-row gathers would cost ~164us/core -- the sort is the trick)
  - per tile: DVE subtract (center broadcast across the slots via a
    stride-0 AP), ACT square + fused row-sum accumulate; the pipeline is
    paced by the feature DMA at ~350 GB/s/core, i.e. the HBM roofline,
    and the small trailing tile drains it quickly
  - each core outputs per-tile partial sums; host sums and divides by B
"""

import numpy as np

import concourse.bacc as bacc
import concourse.bass as bass
import concourse.mybir as mybir
import concourse.tile as tile
from concourse.bass_utils import run_bass_kernel_spmd

B, D, C = 131072, 256, 1000
N_CORES = 8
P = 128

_nc_cache = {}


def _build(slots_list):
    """Per-core graph; tile t holds one slots_list[t]-row block per partition."""
    key = tuple(slots_list)
    if key in _nc_cache:
        return _nc_cache[key]
    T = len(slots_list)
    rows_core = P * sum(slots_list)

    nc = bacc.Bacc()
    feats = nc.declare_dram_parameter(
        "features", [rows_core, D], mybir.dt.float32, isOutput=False
    )
    labels = nc.declare_dram_parameter("labels", [P, T], mybir.dt.int32, isOutput=False)
    centers = nc.declare_dram_parameter(
        "centers", [C, D], mybir.dt.float32, isOutput=False
    )
    out = nc.declare_dram_parameter("out", [P, T], mybir.dt.float32, isOutput=True)

    fall = feats[:]

    with tile.TileContext(nc) as tc:
        with (
            tc.tile_pool(name="lab", bufs=1) as lab_pool,
            tc.tile_pool(name="f", bufs=4) as f_pool,
            tc.tile_pool(name="c", bufs=4) as c_pool,
            tc.tile_pool(name="acc", bufs=1) as acc_pool,
        ):
            lab = lab_pool.tile([P, T], mybir.dt.int32)
            nc.sync.dma_start(out=lab[:], in_=labels[:])
            acc = acc_pool.tile([P, T], mybir.dt.float32)
            rowbase = 0
            for t, slots in enumerate(slots_list):
                f_t = f_pool.tile([P, slots * D], mybir.dt.float32, tag="f")
                nc.sync.dma_start(
                    out=f_t[:].rearrange("p (s d) -> p s d", s=slots),
                    in_=fall[rowbase : rowbase + P * slots, :].rearrange(
                        "(p s) d -> p s d", p=P
                    ),
                )
                c_s = c_pool.tile([P, D], mybir.dt.float32, tag="c")
                nc.gpsimd.indirect_dma_start(
                    out=c_s[:],
                    out_offset=None,
                    in_=centers[:],
                    in_offset=bass.IndirectOffsetOnAxis(ap=lab[:, t : t + 1], axis=0),
                )
                c_b = (
                    c_s[:]
                    .rearrange("p (s d) -> p s d", s=1)
                    .to_broadcast([P, slots, D])
                )
                nc.vector.tensor_tensor(
                    out=f_t[:].rearrange("p (s d) -> p s d", s=slots),
                    in0=f_t[:].rearrange("p (s d) -> p s d", s=slots),
                    in1=c_b,
                    op=mybir.AluOpType.subtract,
                )
                nc.scalar.activation(
                    out=f_t[:],
                    in_=f_t[:],
                    func=mybir.ActivationFunctionType.Square,
                    accum_out=acc[:, t : t + 1],
                )
                rowbase += P * slots
            nc.sync.dma_start(out=out[:], in_=acc[:])
    nc.finalize()
    _nc_cache[key] = nc
    return nc


def _prepare(features, centers, labels):
    features = np.ascontiguousarray(np.asarray(features), dtype=np.float32)
    centers = np.ascontiguousarray(np.asarray(centers), dtype=np.float32)
    labels = np.asarray(labels).astype(np.int32)

    counts = np.bincount(labels, minlength=C)
    full = counts // 16
    rem = counts % 16
    # bulk 16-row blocks; remainders >8 get their own 16-block, <=8 an 8-block
    b16 = full + (rem > 8)
    b8 = ((rem > 0) & (rem <= 8)).astype(np.int64)
    N16, N8 = int(b16.sum()), int(b8.sum())
    group = N_CORES * P
    J16 = max(1, -(-N16 // group))
    J8 = max(1, -(-N8 // group)) if N8 else 0
    slots_list = [16] * J16 + [8] * J8
    rows_core = P * sum(slots_list)

    # block labels per region, class-major; pad blocks use class 0
    lab16 = np.zeros(J16 * group, dtype=np.int32)
    lab16[:N16] = np.repeat(np.arange(C, dtype=np.int32), b16)
    lab8 = np.zeros(J8 * group, dtype=np.int32)
    if N8:
        lab8[:N8] = np.repeat(np.arange(C, dtype=np.int32), b8)

    # global row start of each block position (order: core, tile, partition)
    def region_rows(nblk_core, blk_rows, base_off):
        # block j of core k starts at k*rows_core + base_off + j*blk_rows
        k = np.arange(N_CORES, dtype=np.int64)
        j = np.arange(nblk_core, dtype=np.int64)
        return (
            (k[:, None] * rows_core + base_off + j[None, :] * blk_rows)
            .reshape(-1)
        )

    rs16 = region_rows(J16 * P, 16, 0)
    rs8 = region_rows(J8 * P, 8, J16 * P * 16) if J8 else np.empty(0, np.int64)

    # init every slot with its block's center -> pad rows contribute 0
    fpad = np.empty((N_CORES * rows_core, D), dtype=np.float32)
    if J16:
        rows = (rs16[:, None] + np.arange(16)).ravel()
        fpad[rows] = centers[lab16].repeat(16, axis=0)
    if J8:
        rows = (rs8[:, None] + np.arange(8)).ravel()
        fpad[rows] = centers[lab8].repeat(8, axis=0)

    # scatter real rows
    order = np.argsort(labels)
    labels_sorted = labels[order]
    class_row_start = np.concatenate(([0], np.cumsum(counts)[:-1]))
    start16 = np.concatenate(([0], np.cumsum(b16)[:-1]))
    start8 = np.concatenate(([0], np.cumsum(b8)[:-1]))
    rank = np.arange(B) - class_row_start[labels_sorted]
    cap16 = 16 * b16[labels_sorted]
    in16 = rank < cap16
    dst = np.empty(B, dtype=np.int64)
    blk = start16[labels_sorted[in16]] + rank[in16] // 16
    dst[in16] = rs16[blk] + rank[in16] % 16
    n8m = ~in16
    if n8m.any():
        r8 = rank[n8m] - cap16[n8m]
        dst[n8m] = rs8[start8[labels_sorted[n8m]]] + r8
    fpad[dst] = features[order]

    maps = []
    T = len(slots_list)
    for k in range(N_CORES):
        fs = fpad[k * rows_core : (k + 1) * rows_core]
        lw = np.empty((P, T), dtype=np.int32)
        lw[:, :J16] = lab16[k * J16 * P : (k + 1) * J16 * P].reshape(J16, P).T
        if J8:
            lw[:, J16:] = lab8[k * J8 * P : (k + 1) * J8 * P].reshape(J8, P).T
        maps.append(
            {"features": fs, "labels": np.ascontiguousarray(lw), "centers": centers}
        )
    return maps, slots_list


def run(features, centers, labels, trace=False):
    maps, slots_list = _prepare(features, centers, labels)
    nc = _build(slots_list)
    res = run_bass_kernel_spmd(
        nc, maps, core_ids=list(range(N_CORES)), trace=trace
    )
    total = 0.0
    for r in res.results:
        total += float(np.asarray(r["out"]).astype(np.float64).sum())
    return np.float32(total / B), res


def kernel(features, centers, labels):
    last_err = None
    for _ in range(3):
        try:
            loss, _ = run(features, centers, labels)
            return loss
        except Exception as e:  # noqa: BLE001
            last_err = e
    raise last_err



# revision 5
# speedup vs baseline: 1.2893x; 1.2893x over previous
"""AdaptiveCenterLoss on 8 TRN2 NeuronCores.

loss = mean_i ||features[i] - centers[labels[i]]||^2
     with B=131072, D=256, C=1000.

Strategy (data-parallel, memory-bound):
  - host-side, sort rows by label and pack them into one-label blocks;
    partial blocks are padded with rows equal to that class's center
    (contributing exactly 0 to the sum).  Each class's bulk goes into
    16-row blocks; a remainder of <= 8 rows goes into an 8-row block.
  - features and centers are cast to bf16 on the host: the kernel is
    HBM-bandwidth-bound and the 2e-2 tolerance leaves orders of
    magnitude of headroom (measured rel err ~1e-4), so halving the
    bytes halves the DMA wall.
  - blocks are sharded across 8 cores; per core they form tiles of up
    to 128 blocks (one per partition).  The last tile of each region is
    RAGGED (p < 128 partitions) instead of padding the block count to a
    multiple of 8*128 -- that rounding was ~19%% extra traffic in the
    all-full-tile layout.
  - per tile: ONE [p,1]-index indirect DMA gathers the p needed center
    rows (the DGE consumes one index per partition per call, ~10ns of
    Q7 software time per descriptor); DVE subtracts the broadcast
    center; the square+row-sum is SPLIT between the scalar engine
    (ACT Square+accum on the first slots, 1 elem/cycle dtype-blind)
    and the DVE (tensor_tensor_reduce mult+add on the rest, 2
    elem/cycle at bf16) so neither engine falls behind the bf16 DMA
    pace of ~2.9us/tile.
  - each core outputs per-block partial sums; host sums and divides by B
"""

import numpy as np
import ml_dtypes

import concourse.bacc as bacc
import concourse.bass as bass
import concourse.mybir as mybir
import concourse.tile as tile
from concourse.bass_utils import run_bass_kernel_spmd

B, D, C = 131072, 256, 1000
N_CORES = 8
P = 128

_nc_cache = {}

# ACT takes the first ACT_SLOTS[slots] slots of each block, DVE the rest:
# ACT runs (n+352)/1.2 ns, DVE (bf16 2x) runs n*0.357 ns + the subtract.
ACT_SLOTS = {16: 9, 8: 4}


def _build(tiles):
    """Per-core graph; tiles = ((p, slots), ...), one block/partition."""
    key = tuple(tiles)
    if key in _nc_cache:
        return _nc_cache[key]
    T = len(tiles)
    rows_core = sum(p * s for p, s in tiles)

    nc = bacc.Bacc()
    feats = nc.declare_dram_parameter(
        "features", [rows_core, D], mybir.dt.bfloat16, isOutput=False
    )
    labels = nc.declare_dram_parameter("labels", [P, T], mybir.dt.int32, isOutput=False)
    centers = nc.declare_dram_parameter(
        "centers", [C, D], mybir.dt.bfloat16, isOutput=False
    )
    out = nc.declare_dram_parameter("out", [P, 2 * T], mybir.dt.float32, isOutput=True)

    fall = feats[:]

    with tile.TileContext(nc) as tc:
        with (
            tc.tile_pool(name="lab", bufs=1) as lab_pool,
            tc.tile_pool(name="f", bufs=6) as f_pool,
            tc.tile_pool(name="c", bufs=6) as c_pool,
            tc.tile_pool(name="acc", bufs=1) as acc_pool,
        ):
            lab = lab_pool.tile([P, T], mybir.dt.int32)
            nc.sync.dma_start(out=lab[:], in_=labels[:])
            acc = acc_pool.tile([P, 2 * T], mybir.dt.float32)
            # ragged tiles leave partitions p..127 of their acc columns
            # unwritten; zero them so the final out DMA reads defined data
            nc.vector.memset(acc[:], 0.0)
            rowbase = 0
            for t, (p, slots) in enumerate(tiles):
                f_t = f_pool.tile([P, slots * D], mybir.dt.bfloat16, tag="f")
                nc.sync.dma_start(
                    out=f_t[0:p, :].rearrange("p (s d) -> p s d", s=slots),
                    in_=fall[rowbase : rowbase + p * slots, :].rearrange(
                        "(p s) d -> p s d", p=p
                    ),
                )
                c_s = c_pool.tile([P, D], mybir.dt.bfloat16, tag="c")
                nc.gpsimd.indirect_dma_start(
                    out=c_s[0:p, :],
                    out_offset=None,
                    in_=centers[:],
                    in_offset=bass.IndirectOffsetOnAxis(ap=lab[0:p, t : t + 1], axis=0),
                )
                c_b = (
                    c_s[0:p, :]
                    .rearrange("p (s d) -> p s d", s=1)
                    .to_broadcast([p, slots, D])
                )
                nc.vector.tensor_tensor(
                    out=f_t[0:p, :].rearrange("p (s d) -> p s d", s=slots),
                    in0=f_t[0:p, :].rearrange("p (s d) -> p s d", s=slots),
                    in1=c_b,
                    op=mybir.AluOpType.subtract,
                )
                a = ACT_SLOTS[slots] * D
                nc.scalar.activation(
                    out=f_t[0:p, 0:a],
                    in_=f_t[0:p, 0:a],
                    func=mybir.ActivationFunctionType.Square,
                    accum_out=acc[0:p, 2 * t : 2 * t + 1],
                )
                # (tensor_tensor_reduce crashes on this HW path; STT's
                # accum_out does the same square+row-sum in one DVE op)
                nc.vector.scalar_tensor_tensor(
                    out=f_t[0:p, a : slots * D],
                    in0=f_t[0:p, a : slots * D],
                    scalar=1.0,
                    in1=f_t[0:p, a : slots * D],
                    op0=mybir.AluOpType.mult,
                    op1=mybir.AluOpType.mult,
                    accum_out=acc[0:p, 2 * t + 1 : 2 * t + 2],
                )
                rowbase += p * slots
            nc.sync.dma_start(out=out[:], in_=acc[:])
    nc.finalize()
    _nc_cache[key] = nc
    return nc


def _prepare(features, centers, labels):
    features = np.ascontiguousarray(np.asarray(features), dtype=np.float32)
    centers = np.ascontiguousarray(np.asarray(centers), dtype=np.float32)
    labels = np.asarray(labels).astype(np.int32)

    counts = np.bincount(labels, minlength=C)
    full = counts // 16
    rem = counts % 16
    # bulk 16-row blocks; remainders >8 get their own 16-block, <=8 an 8-block
    b16 = full + (rem > 8)
    b8 = ((rem > 0) & (rem <= 8)).astype(np.int64)
    N16, N8 = int(b16.sum()), int(b8.sum())
    n16c = -(-N16 // N_CORES)
    n8c = -(-N8 // N_CORES) if N8 else 0
    rows_core = 16 * n16c + 8 * n8c

    tiles = []
    t16f, p16 = divmod(n16c, P)
    tiles += [(P, 16)] * t16f + ([(p16, 16)] if p16 else [])
    t8f, p8 = divmod(n8c, P)
    tiles += [(P, 8)] * t8f + ([(p8, 8)] if p8 else [])
    tiles = tuple(tiles)
    T = len(tiles)

    # block labels per region, class-major; pad blocks use class 0
    lab16 = np.zeros(N_CORES * n16c, dtype=np.int32)
    lab16[:N16] = np.repeat(np.arange(C, dtype=np.int32), b16)
    lab8 = np.zeros(N_CORES * n8c, dtype=np.int32)
    if N8:
        lab8[:N8] = np.repeat(np.arange(C, dtype=np.int32), b8)

    # global row start of each block: 16-blocks first within each core
    j16 = np.arange(N_CORES * n16c, dtype=np.int64)
    rs16 = (j16 // n16c) * rows_core + (j16 % n16c) * 16
    if n8c:
        j8 = np.arange(N_CORES * n8c, dtype=np.int64)
        rs8 = (j8 // n8c) * rows_core + 16 * n16c + (j8 % n8c) * 8
    else:
        rs8 = np.empty(0, np.int64)

    # init every slot with its block's center -> pad rows contribute 0
    fpad = np.empty((N_CORES * rows_core, D), dtype=np.float32)
    rows = (rs16[:, None] + np.arange(16)).ravel()
    fpad[rows] = centers[lab16].repeat(16, axis=0)
    if n8c:
        rows = (rs8[:, None] + np.arange(8)).ravel()
        fpad[rows] = centers[lab8].repeat(8, axis=0)

    # scatter real rows
    order = np.argsort(labels)
    labels_sorted = labels[order]
    class_row_start = np.concatenate(([0], np.cumsum(counts)[:-1]))
    start16 = np.concatenate(([0], np.cumsum(b16)[:-1]))
    start8 = np.concatenate(([0], np.cumsum(b8)[:-1]))
    rank = np.arange(B) - class_row_start[labels_sorted]
    cap16 = 16 * b16[labels_sorted]
    in16 = rank < cap16
    dst = np.empty(B, dtype=np.int64)
    blk = start16[labels_sorted[in16]] + rank[in16] // 16
    dst[in16] = rs16[blk] + rank[in16] % 16
    n8m = ~in16
    if n8m.any():
        r8 = rank[n8m] - cap16[n8m]
        dst[n8m] = rs8[start8[labels_sorted[n8m]]] + r8
    fpad[dst] = features[order]

    f16 = fpad.astype(ml_dtypes.bfloat16)
    c16 = centers.astype(ml_dtypes.bfloat16)

    # per-core label tiles [P, T]: column t = classes of that tile's blocks
    maps = []
    for k in range(N_CORES):
        lw = np.zeros((P, T), dtype=np.int32)
        off16 = off8 = 0
        for col, (p, slots) in enumerate(tiles):
            if slots == 16:
                lw[0:p, col] = lab16[k * n16c + off16 : k * n16c + off16 + p]
                off16 += p
            else:
                lw[0:p, col] = lab8[k * n8c + off8 : k * n8c + off8 + p]
                off8 += p
        maps.append(
            {
                "features": f16[k * rows_core : (k + 1) * rows_core],
                "labels": lw,
                "centers": c16,
            }
        )
    return maps, tiles


def run(features, centers, labels, trace=False):
    maps, tiles = _prepare(features, centers, labels)
    nc = _build(tiles)
    res = run_bass_kernel_spmd(
        nc, maps, core_ids=list(range(N_CORES)), trace=trace
    )
    # only (p, t) entries of written partitions are valid; the rest of the
    # out buffer is donated-zero or SBUF garbage -- mask by tile shape
    total = 0.0
    for r in res.results:
        o = np.asarray(r["out"]).astype(np.float64)
        for t, (p, _slots) in enumerate(tiles):
            total += o[0:p, 2 * t] .sum() + o[0:p, 2 * t + 1].sum()
    return np.float32(total / B), res


def kernel(features, centers, labels):
    last_err = None
    for _ in range(3):
        try:
            loss, _ = run(features, centers, labels)
            return loss
        except Exception as e:  # noqa: BLE001
            last_err = e
    raise last_err
